# revision 16
# baseline (speedup 1.0000x reference)
"""LIF neuron (no reset) Trainium2 kernel, v3 (bit-packed spike output).

h_t = 0.5*h_{t-1} + 0.5*x_t ; spike_t = (h_t >= 1.0), x: [T=32, B=64, N=32768] f32.

Sharding: pure data-parallel over batch dim (dim 1) across 8 NeuronCores;
each core scans its [32, 8, 32768] shard over time, with each timestep's
262144-element slab viewed as [128 partitions, 2048].

The kernel is HBM-read-bound (32 MiB/core of fp32 input at ~320 GB/s/core
under 8-core contention ~= 103 us), so v3 minimizes everything else:

  * DVE runs ONLY the scaled recurrence S_t = S_{t-1} + 2^t x_t (one
    scalar_tensor_tensor per step, double-buffered S so the chain never
    waits on other engines' reads).  Scaling the reference's
    h_t = fl(fl(0.5h)+fl(0.5x)) chain by the exact power of two 2^t
    commutes with round-to-nearest, so S is bit-exact vs the reference
    and spike_t = (S_t >= 2^{t+1}).
  * ACT computes sg_t = Sign(S_t - 2^{t+1}) in {-1,0,+1} (bf16).
  * PE accumulates word += (2^k I) @ sg_t into PSUM (fp32, exact: all
    addends are distinct powers of two <= 2^15).
  * After each 16-step half: mask = (word + 65535)/2 is the exact 16-bit
    spike mask (ACT Copy with scale=0.5/bias=32767.5, uint16 out), DMA'd
    out.  Store traffic is 1 MiB/core instead of 8 MiB (u8) / 32 MiB (f32).

Host side unpacks the masks to f32 spikes.  Exact-tie elements
(S_t == 2^{t+1} bit-for-bit, where Sign is 0, probability ~4e-8/element)
decode with <=2 flipped bits; measured 2 mismatches per 8.4M elements --
far inside the 2e-2 rel-err gate.

Measured (reps-slope, 8 cores concurrent, steady-state): ~106-110 us vs
~127-138 us for the v2 uint8 baseline in the same process; pure-load
floor is ~103 us.
"""

import numpy as np

import concourse.bass as bass
import concourse.mybir as mybir
import concourse.tile as tile
from concourse import bacc
from concourse.bass_utils import run_bass_kernel_spmd

T, B, N = 32, 64, 32768
NCORES = 8
B_SH = B // NCORES            # 8 batch rows per core
E = B_SH * N                  # 262144 elements per timestep per core
P = 128                       # SBUF partitions
F = E // P                    # 2048 free-dim columns
FB = 512                      # fp32 columns per PSUM bank

_prog_cache: dict = {}


def build_program_v3(reps: int = 1, bufs: int = 12, sg_bufs: int = 4):
    """Per-core Bass program: x[T, E] f32 -> w[2, E] u16 spike bitmasks.

    w[half, e] bit k = spike at t = 16*half + k.

    reps>1 repeats the whole scan (S re-zeroed each rep) inside a
    hardware For_i loop for wall-clock HW timing: t(reps=K)-t(reps=J)
    ~= (K-J)*kernel_time, cancelling RPC and host-transfer overhead.
    """
    nc = bacc.Bacc()
    x = nc.declare_dram_parameter("x", [T, E], mybir.dt.float32,
                                  isOutput=False)
    ident = nc.declare_dram_parameter("ident", [16 * P, P],
                                      mybir.dt.bfloat16, isOutput=False)
    w = nc.declare_dram_parameter("w", [2, E], mybir.dt.uint16,
                                  isOutput=True)

    with tile.TileContext(nc) as tc:
        with (
            tc.tile_pool(name="xp", bufs=bufs) as xp,
            tc.tile_pool(name="sgp", bufs=sg_bufs) as sgp,
            tc.tile_pool(name="hp", bufs=1) as hp,
            tc.tile_pool(name="wp", bufs=2) as wp,
            tc.tile_pool(name="pp", bufs=1, space="PSUM") as pp,
        ):
            S2 = hp.tile([P, 2, F], mybir.dt.float32, name="S2")
            idt = hp.tile([P, 16, P], mybir.dt.bfloat16, name="idt")
            wlo = pp.tile([P, F], mybir.dt.float32, name="wlo")
            whi = pp.tile([P, F], mybir.dt.float32, name="whi")
            # 16 scaled identities 2^k * I, loaded once per program
            nc.sync.dma_start(idt[:], ident.rearrange("(k p) q -> p k q", p=P))
            # per-step ACT bias constants -(2^(t+1)) as [P,1] columns
            biases = hp.tile([P, T], mybir.dt.float32, name="biases")
            for t in range(T):
                nc.gpsimd.memset(biases[:, t:t + 1], float(-(2.0 ** (t + 1))))

            def drain(word, half):
                # mask = (word + 65535)/2, an exact integer in [0, 65535],
                # drained per PSUM bank on ACT.  Drains are issued OUTSIDE
                # the stall window: ACT is in-order, so a drain emitted
                # right at k==15 waits on that half's final PE matmuls and
                # blocks the next Sign (-26 us/rep).  Instead the lo word
                # drains at t=19 (its matmuls long retired) and the hi
                # word of rep r drains at the top of rep r+1 / in the
                # epilogue for the final rep.
                wu = wp.tile([P, F], mybir.dt.uint16, name="wu", tag="wu")
                for j in range(4):
                    nc.scalar.activation(
                        wu[:, j * FB:(j + 1) * FB],
                        word[:, j * FB:(j + 1) * FB],
                        mybir.ActivationFunctionType.Copy,
                        bias=32767.5, scale=0.5)
                nc.scalar.dma_start(
                    w[half, :].rearrange("(p f) -> p f", p=P), wu[:])

            def body(_i=None):
                # double-buffered S: step t writes S2[:,t%2] reading
                # S2[:,(t+1)%2], so ACT's read of step t never blocks the
                # DVE write of step t+1 (no cross-engine WAR ping-pong).
                # (Quad-buffering adds a 4-step rep-boundary grace window
                # and wins ~10 us/rep in the deep steady-contention
                # regime -- reps {33,161} -- but LOSES ~7 us in the
                # {1,65} single-shot-like regime this kernel is measured
                # in; see memory notes.)
                nc.vector.memset(S2[:, 1, :], 0.0)
                if _i is not None:
                    # prev rep's hi word (garbage on rep 0, overwritten by
                    # every later rep; the final rep drains in the epilogue)
                    drain(whi, 1)
                for t in range(T):
                    s_prev = S2[:, (t + 1) % 2, :]
                    s_cur = S2[:, t % 2, :]
                    xc = xp.tile([P, 1, F], mybir.dt.float32, name="xc",
                                 tag="xc")
                    nc.sync.dma_start(
                        xc[:],
                        x[t:t + 1, :].rearrange("t (p f) -> p t f", p=P))
                    nc.vector.scalar_tensor_tensor(
                        s_cur, xc[:, 0, :], float(2.0 ** t), s_prev,
                        mybir.AluOpType.mult, mybir.AluOpType.add)
                    sg = sgp.tile([P, F], mybir.dt.bfloat16, name="sg",
                                  tag="sg")
                    nc.scalar.activation(
                        sg[:], s_cur, mybir.ActivationFunctionType.Sign,
                        bias=biases[:, t:t + 1], scale=1.0)
                    word = wlo if t < 16 else whi
                    k = t % 16
                    for j in range(4):
                        nc.tensor.matmul(
                            word[:, j * FB:(j + 1) * FB],
                            idt[:, k, :],
                            sg[:, j * FB:(j + 1) * FB],
                            start=(k == 0), stop=(k == 15))
                    if t == 19:
                        drain(wlo, 0)

            if reps == 1:
                body()
            else:
                with tc.For_i(0, reps, 1) as i:
                    body(i)
            drain(whi, 1)
    nc.compile()
    return nc


def make_ident() -> np.ndarray:
    """[16*128, 128] bf16: block k is 2^k * I_128."""
    from ml_dtypes import bfloat16
    ident = np.zeros((16 * P, P), dtype=np.float32)
    eye = np.eye(P, dtype=np.float32)
    for k in range(16):
        ident[k * P:(k + 1) * P] = eye * (2.0 ** k)
    return ident.astype(bfloat16)


def decode_masks(w: np.ndarray) -> np.ndarray:
    """[2, E] u16 bitmasks -> [T, E] f32 spikes."""
    mask = w.astype(np.uint32)                       # [2, E]
    ks = np.arange(16, dtype=np.uint32)
    bits = (mask[:, None, :] >> ks[None, :, None]) & 1   # [2, 16, E]
    return bits.reshape(T, -1).astype(np.float32)    # t = 16*half + k


def run_sharded(x: np.ndarray, nc) -> np.ndarray:
    """Shard [T,B,N] over batch across 8 cores, run, gather + unpack."""
    ident = make_ident()
    in_maps = [
        {
            "x": np.ascontiguousarray(
                x[:, i * B_SH:(i + 1) * B_SH, :]).reshape(T, E),
            "ident": ident,
        }
        for i in range(NCORES)
    ]
    res = run_bass_kernel_spmd(nc, in_maps, list(range(NCORES)))
    out = np.empty((T, B, N), dtype=np.float32)
    for i, r in enumerate(res.results):
        out[:, i * B_SH:(i + 1) * B_SH, :] = decode_masks(
            r["w"]).reshape(T, B_SH, N)
    return out


def build_main_program(reps: int = 1):
    """The shipped configuration (single place to keep test.py in sync).

    bufs=12 beat 14/16 (deeper prefetch congests the contended HBM read
    path); single sync-ring loads beat sync+scalar alternation; the
    PSUM drain on ACT keeps DVE free; double-buffered S wins in the
    measured {1,65} regime (quad only wins under deep sustained
    contention); prefetch depth swept 12>10>8>6>4 monotonically better
    in that regime with 3 worse; with lazy drains the curve still
    falls: 4 < 6 < 8 paired.  Deep steady-contention prefers 12.
    """
    return build_program_v3(reps=reps, bufs=3, sg_bufs=4)


def kernel(x_seq: np.ndarray) -> np.ndarray:
    x = np.asarray(x_seq, dtype=np.float32)
    assert x.shape == (T, B, N), x.shape
    if "main" not in _prog_cache:
        _prog_cache["main"] = build_main_program()
    return run_sharded(x, _prog_cache["main"])


# revision 17
# speedup vs baseline: 1.1774x; 1.1774x over previous
"""LIF neuron (no reset) Trainium2 kernel, v3 (bit-packed spike output).

h_t = 0.5*h_{t-1} + 0.5*x_t ; spike_t = (h_t >= 1.0), x: [T=32, B=64, N=32768] f32.

Sharding: pure data-parallel over batch dim (dim 1) across 8 NeuronCores;
each core scans its [32, 8, 32768] shard over time, with each timestep's
262144-element slab viewed as [128 partitions, 2048].

The kernel is HBM-read-bound (32 MiB/core of fp32 input at ~320 GB/s/core
under 8-core contention ~= 103 us), so v3 minimizes everything else:

  * DVE runs ONLY the scaled recurrence S_t = S_{t-1} + 2^t x_t (one
    scalar_tensor_tensor per step, double-buffered S so the chain never
    waits on other engines' reads).  Scaling the reference's
    h_t = fl(fl(0.5h)+fl(0.5x)) chain by the exact power of two 2^t
    commutes with round-to-nearest, so S is bit-exact vs the reference
    and spike_t = (S_t >= 2^{t+1}).
  * ACT computes sg_t = Sign(S_t - 2^{t+1}) in {-1,0,+1} (bf16).
  * PE accumulates word += (2^k I) @ sg_t into PSUM (fp32, exact: all
    addends are distinct powers of two <= 2^15).
  * After each 16-step half: mask = (word + 65535)/2 is the exact 16-bit
    spike mask (ACT Copy with scale=0.5/bias=32767.5, uint16 out), DMA'd
    out.  Store traffic is 1 MiB/core instead of 8 MiB (u8) / 32 MiB (f32).

Host side unpacks the masks to f32 spikes.  Exact-tie elements
(S_t == 2^{t+1} bit-for-bit, where Sign is 0, probability ~4e-8/element)
decode with <=2 flipped bits; measured 2 mismatches per 8.4M elements --
far inside the 2e-2 rel-err gate.

Measured (reps-slope, 8 cores concurrent, steady-state): ~106-110 us vs
~127-138 us for the v2 uint8 baseline in the same process; pure-load
floor is ~103 us.
"""

import numpy as np

import concourse.bass as bass
import concourse.mybir as mybir
import concourse.tile as tile
from concourse import bacc
from concourse.bass_utils import run_bass_kernel_spmd

T, B, N = 32, 64, 32768
NCORES = 8
B_SH = B // NCORES            # 8 batch rows per core
E = B_SH * N                  # 262144 elements per timestep per core
P = 128                       # SBUF partitions
F = E // P                    # 2048 free-dim columns
FB = 512                      # fp32 columns per PSUM bank

_prog_cache: dict = {}


def build_program_v3(reps: int = 1, bufs: int = 12, sg_bufs: int = 4):
    """Per-core Bass program: x[T, E] f32 -> w[2, E] u16 spike bitmasks.

    w[half, e] bit k = spike at t = 16*half + k.

    reps>1 repeats the whole scan (S re-zeroed each rep) inside a
    hardware For_i loop for wall-clock HW timing: t(reps=K)-t(reps=J)
    ~= (K-J)*kernel_time, cancelling RPC and host-transfer overhead.
    """
    nc = bacc.Bacc()
    x = nc.declare_dram_parameter("x", [T, E], mybir.dt.float32,
                                  isOutput=False)
    ident = nc.declare_dram_parameter("ident", [16 * P, P],
                                      mybir.dt.bfloat16, isOutput=False)
    w = nc.declare_dram_parameter("w", [2, E], mybir.dt.uint16,
                                  isOutput=True)

    with tile.TileContext(nc) as tc:
        with (
            tc.tile_pool(name="xp", bufs=bufs) as xp,
            tc.tile_pool(name="sgp", bufs=sg_bufs) as sgp,
            tc.tile_pool(name="hp", bufs=1) as hp,
            tc.tile_pool(name="wp", bufs=2) as wp,
            tc.tile_pool(name="pp", bufs=1, space="PSUM") as pp,
        ):
            S2 = hp.tile([P, 2, F], mybir.dt.float32, name="S2")
            idt = hp.tile([P, 16, P], mybir.dt.bfloat16, name="idt")
            wlo = pp.tile([P, F], mybir.dt.float32, name="wlo")
            whi = pp.tile([P, F], mybir.dt.float32, name="whi")
            # 16 scaled identities 2^k * I, loaded once per program
            nc.sync.dma_start(idt[:], ident.rearrange("(k p) q -> p k q", p=P))
            # per-step ACT bias constants -(2^(t+1)) as [P,1] columns
            biases = hp.tile([P, T], mybir.dt.float32, name="biases")
            for t in range(T):
                nc.gpsimd.memset(biases[:, t:t + 1], float(-(2.0 ** (t + 1))))

            def drain(word, half):
                # mask = (word + 65535)/2, an exact integer in [0, 65535],
                # drained per PSUM bank on ACT.  Drains are issued OUTSIDE
                # the stall window: ACT is in-order, so a drain emitted
                # right at k==15 waits on that half's final PE matmuls and
                # blocks the next Sign (-26 us/rep).  Instead the lo word
                # drains at t=19 (its matmuls long retired) and the hi
                # word of rep r drains at the top of rep r+1 / in the
                # epilogue for the final rep.
                wu = wp.tile([P, F], mybir.dt.uint16, name="wu", tag="wu")
                for j in range(4):
                    nc.scalar.activation(
                        wu[:, j * FB:(j + 1) * FB],
                        word[:, j * FB:(j + 1) * FB],
                        mybir.ActivationFunctionType.Copy,
                        bias=32767.5, scale=0.5)
                nc.scalar.dma_start(
                    w[half, :].rearrange("(p f) -> p f", p=P), wu[:])

            def body(_i=None):
                # double-buffered S: step t writes S2[:,t%2] reading
                # S2[:,(t+1)%2], so ACT's read of step t never blocks the
                # DVE write of step t+1 (no cross-engine WAR ping-pong).
                # (Quad-buffering adds a 4-step rep-boundary grace window
                # and wins ~10 us/rep in the deep steady-contention
                # regime -- reps {33,161} -- but LOSES ~7 us in the
                # {1,65} single-shot-like regime this kernel is measured
                # in; see memory notes.)
                nc.vector.memset(S2[:, 1, :], 0.0)
                if _i is not None:
                    # prev rep's hi word (garbage on rep 0, overwritten by
                    # every later rep; the final rep drains in the epilogue)
                    drain(whi, 1)
                for t in range(T):
                    s_prev = S2[:, (t + 1) % 2, :]
                    s_cur = S2[:, t % 2, :]
                    xc = xp.tile([P, 1, F], mybir.dt.float32, name="xc",
                                 tag="xc")
                    nc.sync.dma_start(
                        xc[:],
                        x[t:t + 1, :].rearrange("t (p f) -> p t f", p=P))
                    nc.vector.scalar_tensor_tensor(
                        s_cur, xc[:, 0, :], float(2.0 ** t), s_prev,
                        mybir.AluOpType.mult, mybir.AluOpType.add)
                    sg = sgp.tile([P, F], mybir.dt.bfloat16, name="sg",
                                  tag="sg")
                    nc.scalar.activation(
                        sg[:], s_cur, mybir.ActivationFunctionType.Sign,
                        bias=biases[:, t:t + 1], scale=1.0)
                    word = wlo if t < 16 else whi
                    k = t % 16
                    for j in range(4):
                        nc.tensor.matmul(
                            word[:, j * FB:(j + 1) * FB],
                            idt[:, k, :],
                            sg[:, j * FB:(j + 1) * FB],
                            start=(k == 0), stop=(k == 15))
                    if t == 19:
                        drain(wlo, 0)

            if reps == 1:
                body()
            else:
                with tc.For_i(0, reps, 1) as i:
                    body(i)
            drain(whi, 1)
    nc.compile()
    return nc


def make_ident() -> np.ndarray:
    """[16*128, 128] bf16: block k is 2^k * I_128."""
    from ml_dtypes import bfloat16
    ident = np.zeros((16 * P, P), dtype=np.float32)
    eye = np.eye(P, dtype=np.float32)
    for k in range(16):
        ident[k * P:(k + 1) * P] = eye * (2.0 ** k)
    return ident.astype(bfloat16)


def decode_masks(w: np.ndarray) -> np.ndarray:
    """[2, E] u16 bitmasks -> [T, E] f32 spikes."""
    mask = w.astype(np.uint32)                       # [2, E]
    ks = np.arange(16, dtype=np.uint32)
    bits = (mask[:, None, :] >> ks[None, :, None]) & 1   # [2, 16, E]
    return bits.reshape(T, -1).astype(np.float32)    # t = 16*half + k


def run_sharded(x: np.ndarray, nc) -> np.ndarray:
    """Shard [T,B,N] over batch across 8 cores, run, gather + unpack."""
    ident = make_ident()
    in_maps = [
        {
            "x": np.ascontiguousarray(
                x[:, i * B_SH:(i + 1) * B_SH, :]).reshape(T, E),
            "ident": ident,
        }
        for i in range(NCORES)
    ]
    res = run_bass_kernel_spmd(nc, in_maps, list(range(NCORES)))
    out = np.empty((T, B, N), dtype=np.float32)
    for i, r in enumerate(res.results):
        out[:, i * B_SH:(i + 1) * B_SH, :] = decode_masks(
            r["w"]).reshape(T, B_SH, N)
    return out


def build_main_program(reps: int = 1):
    """The shipped configuration (single place to keep test.py in sync).

    bufs=12 beat 14/16 (deeper prefetch congests the contended HBM read
    path); single sync-ring loads beat sync+scalar alternation; the
    PSUM drain on ACT keeps DVE free; double-buffered S wins in the
    measured {1,65} regime (quad only wins under deep sustained
    contention); prefetch depth swept 12>10>8>6>4 monotonically better
    in that regime with 3 worse; with lazy drains the curve still
    falls: 4 < 6 < 8 paired.  Deep steady-contention prefers 12.
    """
    return build_program_v3(reps=reps, bufs=4, sg_bufs=4)


def kernel(x_seq: np.ndarray) -> np.ndarray:
    x = np.asarray(x_seq, dtype=np.float32)
    assert x.shape == (T, B, N), x.shape
    if "main" not in _prog_cache:
        _prog_cache["main"] = build_main_program()
    return run_sharded(x, _prog_cache["main"])


# revision 18
# speedup vs baseline: 1.7524x; 1.4884x over previous
"""LIF neuron (no reset) Trainium2 kernel, v3 (bit-packed spike output).

h_t = 0.5*h_{t-1} + 0.5*x_t ; spike_t = (h_t >= 1.0), x: [T=32, B=64, N=32768] f32.

Sharding: pure data-parallel over batch dim (dim 1) across 8 NeuronCores;
each core scans its [32, 8, 32768] shard over time, with each timestep's
262144-element slab viewed as [128 partitions, 2048].

The kernel is HBM-read-bound (32 MiB/core of fp32 input at ~320 GB/s/core
under 8-core contention ~= 103 us), so v3 minimizes everything else:

  * DVE runs ONLY the scaled recurrence S_t = S_{t-1} + 2^t x_t (one
    scalar_tensor_tensor per step, double-buffered S so the chain never
    waits on other engines' reads).  Scaling the reference's
    h_t = fl(fl(0.5h)+fl(0.5x)) chain by the exact power of two 2^t
    commutes with round-to-nearest, so S is bit-exact vs the reference
    and spike_t = (S_t >= 2^{t+1}).
  * ACT computes sg_t = Sign(S_t - 2^{t+1}) in {-1,0,+1} (bf16).
  * PE accumulates word += (2^k I) @ sg_t into PSUM (fp32, exact: all
    addends are distinct powers of two <= 2^15).
  * After each 16-step half: mask = (word + 65535)/2 is the exact 16-bit
    spike mask (ACT Copy with scale=0.5/bias=32767.5, uint16 out), DMA'd
    out.  Store traffic is 1 MiB/core instead of 8 MiB (u8) / 32 MiB (f32).

Host side unpacks the masks to f32 spikes.  Exact-tie elements
(S_t == 2^{t+1} bit-for-bit, where Sign is 0, probability ~4e-8/element)
decode with <=2 flipped bits; measured 2 mismatches per 8.4M elements --
far inside the 2e-2 rel-err gate.

Final config (bufs=4, double-buffered S, lazy per-bank drains):
E2E draws 80769 / 95548 / 96825 ns across device thermal states vs
125670 ns for the v2 uint8 baseline.  In fresh states the kernel sits
at its double floor: the 8-core-contended load stream (~73-77 us for
32 MiB/core) and the DVE recurrence chain (73 us: 32 steps x
(2048+151)/0.96GHz, the 1x-mode fp32 minimum at one op per element)
are matched, so neither fewer bytes nor fewer DVE cycles is available.
Tuning notes: prefetch depth 4 (the {1,65} regime rewards shallow
per-core demand; deep steady contention prefers 12); drains must sit
outside ACT's in-order stall windows (lo at t=19, hi across the rep
boundary); the per-rep S memset is load-bearing for the boundary
schedule (two removal attempts each cost 10-18 us).
"""

import numpy as np

import concourse.bass as bass
import concourse.mybir as mybir
import concourse.tile as tile
from concourse import bacc
from concourse.bass_utils import run_bass_kernel_spmd

T, B, N = 32, 64, 32768
NCORES = 8
B_SH = B // NCORES            # 8 batch rows per core
E = B_SH * N                  # 262144 elements per timestep per core
P = 128                       # SBUF partitions
F = E // P                    # 2048 free-dim columns
FB = 512                      # fp32 columns per PSUM bank

_prog_cache: dict = {}


def build_program_v3(reps: int = 1, bufs: int = 12, sg_bufs: int = 4):
    """Per-core Bass program: x[T, E] f32 -> w[2, E] u16 spike bitmasks.

    w[half, e] bit k = spike at t = 16*half + k.

    reps>1 repeats the whole scan (S re-zeroed each rep) inside a
    hardware For_i loop for wall-clock HW timing: t(reps=K)-t(reps=J)
    ~= (K-J)*kernel_time, cancelling RPC and host-transfer overhead.
    """
    nc = bacc.Bacc()
    x = nc.declare_dram_parameter("x", [T, E], mybir.dt.float32,
                                  isOutput=False)
    ident = nc.declare_dram_parameter("ident", [16 * P, P],
                                      mybir.dt.bfloat16, isOutput=False)
    w = nc.declare_dram_parameter("w", [2, E], mybir.dt.uint16,
                                  isOutput=True)

    with tile.TileContext(nc) as tc:
        with (
            tc.tile_pool(name="xp", bufs=bufs) as xp,
            tc.tile_pool(name="sgp", bufs=sg_bufs) as sgp,
            tc.tile_pool(name="hp", bufs=1) as hp,
            tc.tile_pool(name="wp", bufs=2) as wp,
            tc.tile_pool(name="pp", bufs=1, space="PSUM") as pp,
        ):
            S2 = hp.tile([P, 2, F], mybir.dt.float32, name="S2")
            idt = hp.tile([P, 16, P], mybir.dt.bfloat16, name="idt")
            wlo = pp.tile([P, F], mybir.dt.float32, name="wlo")
            whi = pp.tile([P, F], mybir.dt.float32, name="whi")
            # 16 scaled identities 2^k * I, loaded once per program
            nc.sync.dma_start(idt[:], ident.rearrange("(k p) q -> p k q", p=P))
            # per-step ACT bias constants -(2^(t+1)) as [P,1] columns
            biases = hp.tile([P, T], mybir.dt.float32, name="biases")
            for t in range(T):
                nc.gpsimd.memset(biases[:, t:t + 1], float(-(2.0 ** (t + 1))))

            def drain(word, half):
                # mask = (word + 65535)/2, an exact integer in [0, 65535],
                # drained per PSUM bank on ACT.  Drains are issued OUTSIDE
                # the stall window: ACT is in-order, so a drain emitted
                # right at k==15 waits on that half's final PE matmuls and
                # blocks the next Sign (-26 us/rep).  Instead the lo word
                # drains at t=19 (its matmuls long retired) and the hi
                # word of rep r drains at the top of rep r+1 / in the
                # epilogue for the final rep.
                wu = wp.tile([P, F], mybir.dt.uint16, name="wu", tag="wu")
                for j in range(4):
                    nc.scalar.activation(
                        wu[:, j * FB:(j + 1) * FB],
                        word[:, j * FB:(j + 1) * FB],
                        mybir.ActivationFunctionType.Copy,
                        bias=32767.5, scale=0.5)
                nc.scalar.dma_start(
                    w[half, :].rearrange("(p f) -> p f", p=P), wu[:])

            def body(_i=None):
                # double-buffered S: step t writes S2[:,t%2] reading
                # S2[:,(t+1)%2], so ACT's read of step t never blocks the
                # DVE write of step t+1 (no cross-engine WAR ping-pong).
                # (Quad-buffering adds a 4-step rep-boundary grace window
                # and wins ~10 us/rep in the deep steady-contention
                # regime -- reps {33,161} -- but LOSES ~7 us in the
                # {1,65} single-shot-like regime this kernel is measured
                # in; see memory notes.)
                nc.vector.memset(S2[:, 1, :], 0.0)
                if _i is not None:
                    # prev rep's hi word (garbage on rep 0, overwritten by
                    # every later rep; the final rep drains in the epilogue)
                    drain(whi, 1)
                for t in range(T):
                    s_prev = S2[:, (t + 1) % 2, :]
                    s_cur = S2[:, t % 2, :]
                    xc = xp.tile([P, 1, F], mybir.dt.float32, name="xc",
                                 tag="xc")
                    nc.sync.dma_start(
                        xc[:],
                        x[t:t + 1, :].rearrange("t (p f) -> p t f", p=P))
                    nc.vector.scalar_tensor_tensor(
                        s_cur, xc[:, 0, :], float(2.0 ** t), s_prev,
                        mybir.AluOpType.mult, mybir.AluOpType.add)
                    sg = sgp.tile([P, F], mybir.dt.bfloat16, name="sg",
                                  tag="sg")
                    nc.scalar.activation(
                        sg[:], s_cur, mybir.ActivationFunctionType.Sign,
                        bias=biases[:, t:t + 1], scale=1.0)
                    word = wlo if t < 16 else whi
                    k = t % 16
                    for j in range(4):
                        nc.tensor.matmul(
                            word[:, j * FB:(j + 1) * FB],
                            idt[:, k, :],
                            sg[:, j * FB:(j + 1) * FB],
                            start=(k == 0), stop=(k == 15))
                    if t == 19:
                        drain(wlo, 0)

            if reps == 1:
                body()
            else:
                with tc.For_i(0, reps, 1) as i:
                    body(i)
            drain(whi, 1)
    nc.compile()
    return nc


def make_ident() -> np.ndarray:
    """[16*128, 128] bf16: block k is 2^k * I_128."""
    from ml_dtypes import bfloat16
    ident = np.zeros((16 * P, P), dtype=np.float32)
    eye = np.eye(P, dtype=np.float32)
    for k in range(16):
        ident[k * P:(k + 1) * P] = eye * (2.0 ** k)
    return ident.astype(bfloat16)


def decode_masks(w: np.ndarray) -> np.ndarray:
    """[2, E] u16 bitmasks -> [T, E] f32 spikes."""
    mask = w.astype(np.uint32)                       # [2, E]
    ks = np.arange(16, dtype=np.uint32)
    bits = (mask[:, None, :] >> ks[None, :, None]) & 1   # [2, 16, E]
    return bits.reshape(T, -1).astype(np.float32)    # t = 16*half + k


def run_sharded(x: np.ndarray, nc) -> np.ndarray:
    """Shard [T,B,N] over batch across 8 cores, run, gather + unpack."""
    ident = make_ident()
    in_maps = [
        {
            "x": np.ascontiguousarray(
                x[:, i * B_SH:(i + 1) * B_SH, :]).reshape(T, E),
            "ident": ident,
        }
        for i in range(NCORES)
    ]
    res = run_bass_kernel_spmd(nc, in_maps, list(range(NCORES)))
    out = np.empty((T, B, N), dtype=np.float32)
    for i, r in enumerate(res.results):
        out[:, i * B_SH:(i + 1) * B_SH, :] = decode_masks(
            r["w"]).reshape(T, B_SH, N)
    return out


def build_main_program(reps: int = 1):
    """The shipped configuration (single place to keep test.py in sync).

    bufs=12 beat 14/16 (deeper prefetch congests the contended HBM read
    path); single sync-ring loads beat sync+scalar alternation; the
    PSUM drain on ACT keeps DVE free; double-buffered S wins in the
    measured {1,65} regime (quad only wins under deep sustained
    contention); prefetch depth swept 12>10>8>6>4 monotonically better
    in that regime with 3 worse; with lazy drains the curve still
    falls: 4 < 6 < 8 paired.  Deep steady-contention prefers 12.
    """
    return build_program_v3(reps=reps, bufs=4, sg_bufs=4)


def kernel(x_seq: np.ndarray) -> np.ndarray:
    x = np.asarray(x_seq, dtype=np.float32)
    assert x.shape == (T, B, N), x.shape
    if "main" not in _prog_cache:
        _prog_cache["main"] = build_main_program()
    return run_sharded(x, _prog_cache["main"])
